# revision 1
# baseline (speedup 1.0000x reference)
"""MDRNN2D (4-direction 2D GRU) Trainium2 Bass kernel.

Sharding: 8 cores = 4 scan directions x 2 batch halves (16 each).
Each core runs a wavefront over the 125 anti-diagonals of its (flipped)
63x63 grid. Hidden state for a diagonal lives in SBUF as (128=hid,
cells*16) with cells ordered by row index i; h_up / h_left of the next
diagonal are 16-column-shifted slices of the previous diagonal's buffer
(block i+1 holds cell i; block 0 and untouched high blocks stay zero and
provide the boundary conditions).

Per cell: r/z/n gate pre-activations are accumulated in PSUM by the
tensor engine (Wx.T@x with K=64 plus Wh.T@h_up and Wh2.T@h_left with
K=128); sigmoid/tanh run on the scalar engine with the per-partition
bias b; the GRU update h = n + z*(0.5*(h_up+h_left) - n) is 3 fused DVE
ops (h_up+h_left runs early on gpsimd, off the critical path).
"""

import os

import numpy as np

B, IN, H_IMG, W_IMG, HID = 32, 64, 64, 64, 128
G = 63            # computed grid is (H-1, W-1)
ND = 2 * G - 1    # number of anti-diagonals
BL = 16           # batch per core
NS = 1            # independent batch streams per core
SB = BL // NS     # batch per stream
STOT = G * G * SB  # columns per stream
TOTAL = NS * STOT

# (i0, i1, ncells, col_offset) per diagonal; cells of diag t are (i, t-i),
# i in [i0, i1], stored as SB consecutive columns per cell, i ascending.
_DIAG = []
_off = 0
for _t in range(ND):
    _i0, _i1 = max(0, _t - (G - 1)), min(_t, G - 1)
    _n = _i1 - _i0 + 1
    _DIAG.append((_i0, _i1, _n, _off))
    _off += _n * SB
assert _off == STOT

_FLIPS = [(False, False), (True, False), (False, True), (True, True)]

_PROG_CACHE = {}
E_ON_POOL = False


def _build_program(mm_dt_name: str):
    import concourse.mybir as mybir
    import concourse.tile as tile
    from concourse import bacc

    f32 = mybir.dt.float32
    mm_dt = getattr(mybir.dt, mm_dt_name)
    AF = mybir.ActivationFunctionType
    OP = mybir.AluOpType

    nc = bacc.Bacc()
    # tensors consumed by the (possibly fp32r) matmuls are declared in the
    # matmul dtype; numpy-side both map to float32 (same bytes)
    xd = nc.declare_dram_parameter("xd", [IN, TOTAL], mm_dt, isOutput=False)
    wxp = nc.declare_dram_parameter("wx", [IN, 3 * HID], mm_dt, isOutput=False)
    whp = nc.declare_dram_parameter("wh", [HID, 3 * HID], mm_dt, isOutput=False)
    wh2p = nc.declare_dram_parameter("wh2", [HID, 3 * HID], mm_dt, isOutput=False)
    bp = nc.declare_dram_parameter("bias", [HID, 3], f32, isOutput=False)
    od = nc.declare_dram_parameter("od", [HID, TOTAL], f32, isOutput=True)

    def as32(v):
        return v if mm_dt is f32 else v.bitcast(f32)

    with tile.TileContext(nc) as tc:
        with (
            tc.tile_pool(name="const", bufs=1) as cpool,
            tc.tile_pool(name="hbuf", bufs=1) as hpool,
            tc.tile_pool(name="xin", bufs=4) as xpool,
            tc.tile_pool(name="ps", bufs=2, space="PSUM") as ppool,
            tc.tile_pool(name="work", bufs=3) as wpool,
        ):
            wx_t = cpool.tile([IN, 3 * HID], mm_dt, tag="wx")
            wh_t = cpool.tile([HID, 3 * HID], mm_dt, tag="wh")
            wh2_t = cpool.tile([HID, 3 * HID], mm_dt, tag="wh2")
            b_t = cpool.tile([HID, 3], f32, tag="b")
            nc.sync.dma_start(out=wx_t[:], in_=wxp[:])
            nc.sync.dma_start(out=wh_t[:], in_=whp[:])
            nc.sync.dma_start(out=wh2_t[:], in_=wh2p[:])
            nc.sync.dma_start(out=b_t[:], in_=bp[:])

            hbufs = []
            for k in range(2 * NS):
                hb = hpool.tile([HID, (G + 2) * SB], mm_dt, tag=f"h{k}", name=f"hring{k}")
                hbufs.append(hb)
                nc.vector.memset(as32(hb[:]), 0.0)

            wxg = [wx_t[:, g * HID:(g + 1) * HID] for g in range(3)]
            whg = [wh_t[:, g * HID:(g + 1) * HID] for g in range(3)]
            wh2g = [wh2_t[:, g * HID:(g + 1) * HID] for g in range(3)]

            for t in range(ND):
                i0, i1, n, off = _DIAG[t]
                for s in range(NS):
                    cur = hbufs[s * 2 + (t % 2)]
                    prev = hbufs[s * 2 + ((t - 1) % 2)]
                    base = s * STOT + off
                    cols = n * SB
                    x_t = xpool.tile([IN, cols], mm_dt, tag="xt")
                    nc.sync.dma_start(out=x_t[:], in_=xd[:, base:base + cols])
                    chunks = [(0, n)] if n <= 32 else [(0, n // 2), (n // 2, n)]
                    for (c0, c1) in chunks:
                        w = (c1 - c0) * SB
                        xs = c0 * SB
                        pr = ppool.tile([HID, w], f32, tag="pr")
                        pz = ppool.tile([HID, w], f32, tag="pz")
                        png = ppool.tile([HID, w], f32, tag="png")
                        pnx = ppool.tile([HID, w], f32, tag="pnx")
                        xin = x_t[:, xs:xs + w]
                        # x-only matmuls first: no dependency on the
                        # previous diagonal, so PE stays busy during the
                        # previous step's DVE tail
                        nc.tensor.matmul(pnx[:], wxg[2], xin, start=True, stop=True)
                        nc.tensor.matmul(pr[:], wxg[0], xin, start=True, stop=False)
                        nc.tensor.matmul(pz[:], wxg[1], xin, start=True, stop=False)

                        ob = (i0 + 1 + c0) * SB
                        up = (i0 + c0) * SB
                        lf = (i0 + 1 + c0) * SB
                        h_up = prev[:, up:up + w]
                        h_left = prev[:, lf:lf + w]

                        # s = h_up + h_left on gpsimd: inputs ready at
                        # step start, hides under the matmuls
                        s_t = wpool.tile([HID, w], f32, tag="s")
                        nc.gpsimd.tensor_add(s_t[:], as32(h_up), as32(h_left))

                        nc.tensor.matmul(pr[:], whg[0], h_up, start=False, stop=False)
                        nc.tensor.matmul(pr[:], wh2g[0], h_left, start=False, stop=True)
                        nc.tensor.matmul(png[:], whg[2], h_up, start=True, stop=False)
                        nc.tensor.matmul(png[:], wh2g[2], h_left, start=False, stop=True)
                        nc.tensor.matmul(pz[:], whg[1], h_up, start=False, stop=False)
                        nc.tensor.matmul(pz[:], wh2g[1], h_left, start=False, stop=True)

                        r_t = wpool.tile([HID, w], f32, tag="r")
                        nc.scalar.activation(r_t[:], pr[:], AF.Sigmoid, bias=b_t[:, 0:1])

                        # n = tanh(r*png + pnx + b_n)
                        v_t = wpool.tile([HID, w], f32, tag="v")
                        nc.vector.tensor_mul(v_t[:], r_t[:], png[:])
                        w_t = wpool.tile([HID, w], f32, tag="w")
                        nc.vector.tensor_add(w_t[:], v_t[:], pnx[:])
                        n_t = wpool.tile([HID, w], f32, tag="n")
                        nc.scalar.activation(n_t[:], w_t[:], AF.Tanh, bias=b_t[:, 2:3])
                        z_t = wpool.tile([HID, w], f32, tag="z")
                        nc.scalar.activation(z_t[:], pz[:], AF.Sigmoid, bias=b_t[:, 1:2])

                        # h = n + z*(0.5*s - n) = 0.5*(z*(s - 2n)) + n
                        q_t = wpool.tile([HID, w], f32, tag="q")
                        nc.vector.scalar_tensor_tensor(
                            q_t[:], n_t[:], -2.0, s_t[:], OP.mult, OP.add)
                        e_t = wpool.tile([HID, w], f32, tag="e")
                        if E_ON_POOL:
                            nc.gpsimd.tensor_mul(e_t[:], z_t[:], q_t[:])
                        else:
                            nc.vector.tensor_mul(e_t[:], z_t[:], q_t[:])
                        # write rounds to the matmul dtype (verifier
                        # requires fp32r-rounded producers)
                        nc.vector.scalar_tensor_tensor(
                            cur[:, ob:ob + w], e_t[:], 0.5, n_t[:], OP.mult, OP.add)

                        nc.sync.dma_start(
                            out=od[:, base + xs:base + xs + w],
                            in_=as32(cur[:, ob:ob + w]))

    nc.finalize()
    return nc


def _host_prep(x, Wx, Wh, Wh2, b):
    """Build per-core input maps (8 cores = 4 dirs x 2 batch halves)."""
    xr = np.ascontiguousarray(np.transpose(x, (2, 3, 0, 1))[:G, :G])  # (G,G,B,IN)
    in_maps = []
    for d, (fy, fx) in enumerate(_FLIPS):
        xg = xr[::-1] if fy else xr
        xg = xg[:, ::-1] if fx else xg
        for half in range(2):
            xdiag = np.empty((IN, TOTAL), np.float32)
            for s in range(NS):
                b0 = half * BL + s * SB
                xh = xg[:, :, b0:b0 + SB]          # (G,G,SB,IN)
                for t in range(ND):
                    i0, i1, n, off = _DIAG[t]
                    ii = np.arange(i0, i1 + 1)
                    blk = xh[ii, t - ii]           # (n, SB, IN)
                    xdiag[:, s * STOT + off:s * STOT + off + n * SB] = \
                        blk.reshape(n * SB, IN).T
            bias = np.ascontiguousarray(b[d].reshape(3, HID).T)  # (HID,3)
            in_maps.append({
                "xd": xdiag,
                "wx": np.ascontiguousarray(Wx[d]),
                "wh": np.ascontiguousarray(Wh[d]),
                "wh2": np.ascontiguousarray(Wh2[d]),
                "bias": bias,
            })
    return in_maps


def _host_gather(results):
    out_map = np.ones((4, H_IMG, W_IMG, B, HID), np.float32)
    for d, (fy, fx) in enumerate(_FLIPS):
        o = np.empty((G, G, B, HID), np.float32)
        for half in range(2):
            od = results[d * 2 + half]["od"]  # (HID, TOTAL)
            for s in range(NS):
                b0 = half * BL + s * SB
                for t in range(ND):
                    i0, i1, n, off = _DIAG[t]
                    sl = od[:, s * STOT + off:s * STOT + off + n * SB]
                    blk = sl.T.reshape(n, SB, HID)
                    ii = np.arange(i0, i1 + 1)
                    o[ii, t - ii, b0:b0 + SB] = blk
        o = o[::-1] if fy else o
        o = o[:, ::-1] if fx else o
        oy, ox = (1 if fy else 0), (1 if fx else 0)
        out_map[d, oy:oy + G, ox:ox + G] = o
    return np.ascontiguousarray(np.transpose(out_map, (3, 4, 0, 1, 2)))


def kernel(x, Wx, Wh, Wh2, b):
    from concourse.bass_utils import run_bass_kernel_spmd

    mm_dt = os.environ.get("MDRNN_MM_DT", "float32r")
    if mm_dt not in _PROG_CACHE:
        _PROG_CACHE[mm_dt] = _build_program(mm_dt)
    nc = _PROG_CACHE[mm_dt]

    in_maps = _host_prep(
        np.asarray(x, np.float32), np.asarray(Wx, np.float32),
        np.asarray(Wh, np.float32), np.asarray(Wh2, np.float32),
        np.asarray(b, np.float32))
    trace = os.environ.get("MDRNN_TRACE", "0") == "1"
    res = run_bass_kernel_spmd(nc, in_maps, list(range(8)), trace=trace)
    out = _host_gather(res.results)
    if trace:
        kernel.last_exec_time_ns = res.exec_time_ns
        kernel.last_profile = res
    return out

